# revision 18
# baseline (speedup 1.0000x reference)
"""Trainium2 Bass kernel for the LC0-style attention policy head.

Computation (per board of 64 squares):
  q = x @ Wq + bq; k = x @ Wk + bk            (P = 128)
  logits_ft = q @ k^T * P**-0.5               (64 x 64)
  logits_64[f, p] = logits_ft[f, to_sq[f, p]] masked to -1e9 where invalid
  up = (x @ Wu + bu) masked to -1e9 except rank-7 rows (48..55)
  out = concat([logits_64, up], -1).reshape(64 * 73)

Sharding: pure data parallel, batch 1024 -> 8 cores x 128.

Key layout ideas:
  * x arrives [tokens, d]; projections need d on partitions, so x blocks are
    transposed on the PE (f32 PE transpose is exact).
  * Attention matmuls produce P^T tiles [to_sq, (board, from_sq)] packed in
    SBUF across the whole core batch.
  * The to-square gather is a per-from-square selection matrix matmul:
    out[b, planes] = PT[:, b*64+f].T @ Sel_f, with boards on PSUM partitions.
    A constant ones-row appended to PT plus a bias row in Sel folds the
    "-1e9 where invalid" mask into the same matmul.
  * The underpromotion head runs on promo tokens only (8 per board), gathered
    during phase 1 into a [d, promo-token] buffer.
"""

import sys

sys.path.insert(0, "/opt/trn_rl_repo")

from contextlib import ExitStack

import numpy as np

import concourse.bass as bass
import concourse.mybir as mybir
import concourse.tile as tile
from concourse import bacc, bass_utils
from concourse._compat import with_exitstack

F32 = mybir.dt.float32
F32R = mybir.dt.float32r
USE_F32R_PROJ = True
PROJ_DT = F32R if USE_F32R_PROJ else F32

N_CORES = 8
B_TOTAL = 1024
B_SHARD = B_TOTAL // N_CORES          # 128 boards per core
GROUP_B = 8                            # boards per phase-1 group
N_GROUPS = B_SHARD // GROUP_B          # 16
TOK_G = GROUP_B * 64                   # 512 tokens per group
D = 1024
P = 128
SCALE = float(P) ** -0.5
NEG_INF = -1e9


def _build_tables():
    to_sq = np.zeros((64, 64), np.int32)
    valid = np.zeros((64, 64), bool)
    dirs = [(1, 0), (1, 1), (0, 1), (-1, 1), (-1, 0), (-1, -1), (0, -1), (1, -1)]
    knights = [(2, 1), (1, 2), (-1, 2), (-2, 1), (-2, -1), (-1, -2), (1, -2), (2, -1)]
    for sq in range(64):
        r, c = divmod(sq, 8)
        for d, (dr, dc) in enumerate(dirs):
            for dist in range(1, 8):
                p = d * 7 + (dist - 1)
                nr, nc = r + dr * dist, c + dc * dist
                if 0 <= nr < 8 and 0 <= nc < 8:
                    to_sq[sq, p] = nr * 8 + nc
                    valid[sq, p] = True
        for i, (dr, dc) in enumerate(knights):
            nr, nc = r + dr, c + dc
            if 0 <= nr < 8 and 0 <= nc < 8:
                to_sq[sq, 56 + i] = nr * 8 + nc
                valid[sq, 56 + i] = True
    return to_sq, valid


def _build_sel65():
    """sel[t, f*73 + p]: selection matrix plus bias row (row 64)."""
    to_sq, valid = _build_tables()
    sel = np.zeros((65, 64 * 73), np.float32)
    for f in range(64):
        for p in range(64):
            if valid[f, p]:
                sel[to_sq[f, p], f * 73 + p] = 1.0
            else:
                sel[64, f * 73 + p] = NEG_INF
        for j in range(9):
            if not (48 <= f < 56):
                sel[64, f * 73 + 64 + j] = NEG_INF
    return sel


@with_exitstack
def policy_head_kernel(ctx: ExitStack, tc: tile.TileContext, out, x, wq, wk, wu,
                       sbq, bk1, bu72, sel, ident):
    nc = tc.nc
    xt_flat = x.flatten_outer_dims()      # [8192, 1024] tokens x d

    const = ctx.enter_context(tc.tile_pool(name="const", bufs=1))
    big = ctx.enter_context(tc.tile_pool(name="big", bufs=1))

    # ---- constants / weights in SBUF
    wq_sb = const.tile([128, D], F32, tag="wq_sb")       # 8 chunks [128,128]
    wk_sb = const.tile([128, D], F32, tag="wk_sb")
    wu_sb = const.tile([128, 72], F32, tag="wu_sb")      # 8 chunks [128,9]
    idt = const.tile([128, 128], F32, tag="idt")
    sbq_sb = const.tile([128, 1], F32, tag="sbq_sb")
    bk_sb = const.tile([128, 1], F32, tag="bk_sb")
    bu_sb = const.tile([128, 72], F32, tag="bu_sb")
    sel_sb = const.tile([65, 64 * 73], F32, tag="sel_sb")
    for c in range(8):
        nc.sync.dma_start(wq_sb[:, c * 128:(c + 1) * 128],
                          wq[c * 128:(c + 1) * 128, :])
        nc.sync.dma_start(wk_sb[:, c * 128:(c + 1) * 128],
                          wk[c * 128:(c + 1) * 128, :])
        nc.sync.dma_start(wu_sb[:, c * 9:(c + 1) * 9],
                          wu[c * 128:(c + 1) * 128, :])
    if USE_F32R_PROJ:
        wq_r = const.tile([128, D], PROJ_DT, tag="wq_r")
        wk_r = const.tile([128, D], PROJ_DT, tag="wk_r")
        nc.vector.tensor_copy(wq_r[:], wq_sb[:])
        nc.vector.tensor_copy(wk_r[:], wk_sb[:])
    else:
        wq_r, wk_r = wq_sb, wk_sb
    nc.sync.dma_start(idt[:], ident[:])
    nc.sync.dma_start(sbq_sb[:], sbq[:])
    nc.sync.dma_start(bk_sb[:], bk1[:])
    nc.sync.dma_start(bu_sb[:], bass.AP(bu72.tensor, 0, [[0, 128], [1, 72]]))
    nc.sync.dma_start(sel_sb[:], sel[:])

    # ---- long-lived accumulators
    # PT: [to_sq(+ones row), board*64+from_sq] across the whole core batch
    pt = big.tile([65, B_SHARD * 64], F32, tag="pt")
    # promoxt layout: per chunk c (free offset c*1024): [d_in_chunk=128part,
    #   g*64 + b_l*8 + s_l]
    promoxt = big.tile([128, 8 * B_SHARD * 8], F32, tag="promoxt")
    staging = big.tile([128, 64 * 73], F32, tag="staging")

    nc.gpsimd.memset(pt[64:65, :], 1.0)

    # ================= phase 1: projections + attention =================
    with (
        tc.tile_pool(name="xin", bufs=8) as xin_pool,
        tc.tile_pool(name="xtp", bufs=4, space="PSUM") as xtp_pool,
        tc.tile_pool(name="xt", bufs=2) as xt_pool,
        tc.tile_pool(name="qkp", bufs=1, space="PSUM") as qkp_pool,
        tc.tile_pool(name="qkt", bufs=2) as qkt_pool,
        tc.tile_pool(name="app", bufs=2, space="PSUM") as app_pool,
    ):
        for g in range(N_GROUPS):
            # load 512 tokens (8 boards) as 4 tiles [128 tok, 1024]
            xins = []
            for t in range(4):
                xin = xin_pool.tile([128, D], F32, tag="xin")
                tok0 = g * TOK_G + t * 128
                nc.sync.dma_start(xin[:], xt_flat[tok0:tok0 + 128, :])
                xins.append(xin)

            # transpose to xt chunks [d_chunk=128, 512 tok]
            xts = []
            for c in range(8):
                xtp = xtp_pool.tile([128, TOK_G], F32, tag="xtp")
                for t in range(4):
                    nc.tensor.transpose(
                        xtp[:, t * 128:(t + 1) * 128],
                        xins[t][:, c * 128:(c + 1) * 128],
                        idt[:],
                    )
                xt = xt_pool.tile([128, TOK_G], PROJ_DT, tag=f"xt{c}")
                if c % 2 == 0:
                    nc.vector.tensor_copy(xt[:], xtp[:])
                else:
                    nc.scalar.copy(xt[:], xtp[:])
                xts.append(xt)

            # promo-token slices of xT (columns b_l*64 + 48 + s)
            for c in range(8):
                src = xts[c][:].rearrange("p (b s) -> p b s", b=GROUP_B)[:, :, 48:56]
                dst = promoxt[:].rearrange(
                    "p (c g b s) -> p c g b s", c=8, g=N_GROUPS, b=GROUP_B
                )[:, c, g, :, :]
                nc.gpsimd.tensor_copy(dst, src)

            # q/k projections: [p=128, tok]
            qp = qkp_pool.tile([128, TOK_G], F32, tag="qp")
            kp = qkp_pool.tile([128, TOK_G], F32, tag="kp")
            for c in range(8):
                nc.tensor.matmul(qp[:], wq_r[:, c * 128:(c + 1) * 128],
                                 xts[c][:], start=(c == 0), stop=(c == 7))
            for c in range(8):
                nc.tensor.matmul(kp[:], wk_r[:, c * 128:(c + 1) * 128],
                                 xts[c][:], start=(c == 0), stop=(c == 7))
            qt = qkt_pool.tile([128, TOK_G], F32, tag="qt")
            kt = qkt_pool.tile([128, TOK_G], F32, tag="kt")
            nc.vector.tensor_scalar(qt[:], qp[:], SCALE, sbq_sb[:],
                                    op0=mybir.AluOpType.mult,
                                    op1=mybir.AluOpType.add)
            nc.scalar.activation(kt[:], kp[:],
                                 mybir.ActivationFunctionType.Identity,
                                 bias=bk_sb[:], scale=1.0)

            # attention: P^T[t, f] per board
            ap = app_pool.tile([64, TOK_G], F32, tag="ap")
            for b in range(GROUP_B):
                nc.tensor.matmul(ap[:, b * 64:(b + 1) * 64],
                                 kt[:, b * 64:(b + 1) * 64],
                                 qt[:, b * 64:(b + 1) * 64],
                                 start=True, stop=True)
            nc.vector.tensor_copy(pt[0:64, g * TOK_G:(g + 1) * TOK_G], ap[:])

    # ================= phase 2: gather + underpromotions =================
    with (
        tc.tile_pool(name="gp", bufs=3, space="PSUM") as gp_pool,
        tc.tile_pool(name="up", bufs=1, space="PSUM") as up_pool,
    ):
        # underpromotion head first: up[b, s*9+j] = x[b,48+s,:] @ Wu[:,j]
        up = up_pool.tile([128, 72], F32, tag="up")
        for s_l in range(8):
            for c in range(8):
                lhs = promoxt[:].rearrange(
                    "p (c bs s) -> p c bs s", c=8, s=8)[:, c, :, s_l]
                nc.tensor.matmul(up[:, s_l * 9:(s_l + 1) * 9], lhs,
                                 wu_sb[:, c * 9:(c + 1) * 9],
                                 start=(c == 0), stop=(c == 7))

        f_per = 6
        n_fg = (64 + f_per - 1) // f_per          # 11 groups (last has 4)
        for fg in range(n_fg):
            f0 = fg * f_per
            nf = min(f_per, 64 - f0)
            gp = gp_pool.tile([128, f_per * 73], F32, tag="gp")
            for f_l in range(nf):
                f = f0 + f_l
                lhs = pt[:].rearrange("t (b f) -> t b f", f=64)[:, :, f]
                nc.tensor.matmul(gp[:, f_l * 73:(f_l + 1) * 73], lhs,
                                 sel_sb[:, f * 73:(f + 1) * 73],
                                 start=True, stop=True)
            # merge u (+bu) into the promo columns of this psum block
            pf0, pf1 = max(f0, 48), min(f0 + nf, 56)
            if pf0 < pf1:
                dst = gp[:].rearrange("b (f p) -> b f p", p=73)[
                    :, pf0 - f0:pf1 - f0, 64:73]
                src = up[:].rearrange("b (s j) -> b s j", s=8)[
                    :, pf0 - 48:pf1 - 48, :]
                b_ap = bu_sb[:].rearrange("b (s j) -> b s j", s=8)[
                    :, pf0 - 48:pf1 - 48, :]
                nc.vector.tensor_tensor(dst, src, b_ap, op=mybir.AluOpType.add)
            nc.vector.tensor_copy(staging[:, f0 * 73:(f0 + nf) * 73],
                                  gp[:, 0:nf * 73])
            nc.sync.dma_start(out[:, f0 * 73:(f0 + nf) * 73],
                              staging[:, f0 * 73:(f0 + nf) * 73])


def build_nc():
    nc = bacc.Bacc("TRN2", target_bir_lowering=False, debug=False)
    x = nc.dram_tensor("x", [B_SHARD, 64, D], F32, kind="ExternalInput").ap()
    wq = nc.dram_tensor("wq", [D, P], F32, kind="ExternalInput").ap()
    wk = nc.dram_tensor("wk", [D, P], F32, kind="ExternalInput").ap()
    wu = nc.dram_tensor("wu", [D, 9], F32, kind="ExternalInput").ap()
    sbq = nc.dram_tensor("sbq", [P, 1], F32, kind="ExternalInput").ap()
    bk1 = nc.dram_tensor("bk1", [P, 1], F32, kind="ExternalInput").ap()
    bu72 = nc.dram_tensor("bu72", [1, 72], F32, kind="ExternalInput").ap()
    sel = nc.dram_tensor("sel", [65, 64 * 73], F32, kind="ExternalInput").ap()
    ident = nc.dram_tensor("ident", [128, 128], F32, kind="ExternalInput").ap()
    out = nc.dram_tensor("out", [B_SHARD, 64 * 73], F32,
                         kind="ExternalOutput").ap()
    with tile.TileContext(nc) as tc:
        policy_head_kernel(tc, out, x, wq, wk, wu, sbq, bk1, bu72, sel, ident)
    nc.compile()
    return nc


_NC_CACHE = None


def _get_nc():
    global _NC_CACHE
    if _NC_CACHE is None:
        _NC_CACHE = build_nc()
    return _NC_CACHE


def make_in_maps(x, Wq, bq, Wk, bk, Wu, bu):
    sel = _build_sel65()
    ident = np.eye(128, dtype=np.float32)
    sbq = (np.asarray(bq, np.float32) * SCALE).reshape(P, 1)
    bk1 = np.asarray(bk, np.float32).reshape(P, 1)
    bu72 = np.tile(np.asarray(bu, np.float32), 8).reshape(1, 72)
    xs = np.ascontiguousarray(np.asarray(x, np.float32)).reshape(
        N_CORES, B_SHARD, 64, D)
    base = dict(wq=np.asarray(Wq, np.float32), wk=np.asarray(Wk, np.float32),
                wu=np.asarray(Wu, np.float32), sbq=sbq, bk1=bk1, bu72=bu72,
                sel=sel, ident=ident)
    return [dict(base, x=xs[c]) for c in range(N_CORES)]


def kernel(x, Wq, bq, Wk, bk, Wu, bu):
    nc = _get_nc()
    in_maps = make_in_maps(x, Wq, bq, Wk, bk, Wu, bu)
    res = bass_utils.run_bass_kernel_spmd(nc, in_maps,
                                          core_ids=list(range(N_CORES)))
    return np.concatenate([r["out"] for r in res.results], axis=0)


# revision 21
# speedup vs baseline: 1.0491x; 1.0491x over previous
"""Trainium2 Bass kernel for the LC0-style attention policy head.

Computation (per board of 64 squares):
  q = x @ Wq + bq; k = x @ Wk + bk            (P = 128)
  logits_ft = q @ k^T * P**-0.5               (64 x 64)
  logits_64[f, p] = logits_ft[f, to_sq[f, p]] masked to -1e9 where invalid
  up = (x @ Wu + bu) masked to -1e9 except rank-7 rows (48..55)
  out = concat([logits_64, up], -1).reshape(64 * 73)

Sharding: pure data parallel, batch 1024 -> 8 cores x 128.

Key layout ideas:
  * x arrives [tokens, d]; projections need d on partitions, so x blocks are
    transposed on the PE (f32 PE transpose is exact).
  * Attention matmuls produce P^T tiles [to_sq, (board, from_sq)] packed in
    SBUF across the whole core batch.
  * The to-square gather is a per-from-square selection matrix matmul:
    out[b, planes] = PT[:, b*64+f].T @ Sel_f, with boards on PSUM partitions.
    A constant ones-row appended to PT plus a bias row in Sel folds the
    "-1e9 where invalid" mask into the same matmul.
  * The underpromotion head runs on promo tokens only (8 per board), gathered
    during phase 1 into a [d, promo-token] buffer.
"""

import sys

sys.path.insert(0, "/opt/trn_rl_repo")

from contextlib import ExitStack

import numpy as np

import concourse.bass as bass
import concourse.mybir as mybir
import concourse.tile as tile
from concourse import bacc, bass_utils
from concourse._compat import with_exitstack

F32 = mybir.dt.float32
F32R = mybir.dt.float32r
USE_F32R_PROJ = True
PROJ_DT = F32R if USE_F32R_PROJ else F32

N_CORES = 8
B_TOTAL = 1024
B_SHARD = B_TOTAL // N_CORES          # 128 boards per core
GROUP_B = 8                            # boards per phase-1 group
N_GROUPS = B_SHARD // GROUP_B          # 16
TOK_G = GROUP_B * 64                   # 512 tokens per group
D = 1024
P = 128
SCALE = float(P) ** -0.5
NEG_INF = -1e9


def _build_tables():
    to_sq = np.zeros((64, 64), np.int32)
    valid = np.zeros((64, 64), bool)
    dirs = [(1, 0), (1, 1), (0, 1), (-1, 1), (-1, 0), (-1, -1), (0, -1), (1, -1)]
    knights = [(2, 1), (1, 2), (-1, 2), (-2, 1), (-2, -1), (-1, -2), (1, -2), (2, -1)]
    for sq in range(64):
        r, c = divmod(sq, 8)
        for d, (dr, dc) in enumerate(dirs):
            for dist in range(1, 8):
                p = d * 7 + (dist - 1)
                nr, nc = r + dr * dist, c + dc * dist
                if 0 <= nr < 8 and 0 <= nc < 8:
                    to_sq[sq, p] = nr * 8 + nc
                    valid[sq, p] = True
        for i, (dr, dc) in enumerate(knights):
            nr, nc = r + dr, c + dc
            if 0 <= nr < 8 and 0 <= nc < 8:
                to_sq[sq, 56 + i] = nr * 8 + nc
                valid[sq, 56 + i] = True
    return to_sq, valid


def _build_sel65():
    """sel[t, f*73 + p]: selection matrix plus bias row (row 64)."""
    to_sq, valid = _build_tables()
    sel = np.zeros((65, 64 * 73), np.float32)
    for f in range(64):
        for p in range(64):
            if valid[f, p]:
                sel[to_sq[f, p], f * 73 + p] = 1.0
            else:
                sel[64, f * 73 + p] = NEG_INF
        for j in range(9):
            if not (48 <= f < 56):
                sel[64, f * 73 + 64 + j] = NEG_INF
    return sel


@with_exitstack
def policy_head_kernel(ctx: ExitStack, tc: tile.TileContext, out, x, wq, wk, wu,
                       sbq, bk1, bu72, sel, ident):
    nc = tc.nc
    xt_flat = x.flatten_outer_dims()      # [8192, 1024] tokens x d

    const = ctx.enter_context(tc.tile_pool(name="const", bufs=1))
    big = ctx.enter_context(tc.tile_pool(name="big", bufs=1))

    # ---- constants / weights in SBUF
    wq_sb = const.tile([128, D], F32, tag="wq_sb")       # 8 chunks [128,128]
    wk_sb = const.tile([128, D], F32, tag="wk_sb")
    wu_sb = const.tile([128, 72], F32, tag="wu_sb")      # 8 chunks [128,9]
    idt = const.tile([128, 128], F32, tag="idt")
    sbq_sb = const.tile([128, 1], F32, tag="sbq_sb")
    bk_sb = const.tile([128, 1], F32, tag="bk_sb")
    bu_sb = const.tile([128, 72], F32, tag="bu_sb")
    sel_sb = const.tile([65, 64 * 73], F32, tag="sel_sb")
    nc.sync.dma_start(wq_sb[:].rearrange("p (c n) -> p c n", c=8),
                      wq[:].rearrange("(c p) n -> p c n", p=128))
    nc.sync.dma_start(wk_sb[:].rearrange("p (c n) -> p c n", c=8),
                      wk[:].rearrange("(c p) n -> p c n", p=128))
    nc.sync.dma_start(wu_sb[:].rearrange("p (c n) -> p c n", c=8),
                      wu[:].rearrange("(c p) n -> p c n", p=128))
    if USE_F32R_PROJ:
        wq_r = const.tile([128, D], PROJ_DT, tag="wq_r")
        wk_r = const.tile([128, D], PROJ_DT, tag="wk_r")
        nc.vector.tensor_copy(wq_r[:], wq_sb[:])
        nc.vector.tensor_copy(wk_r[:], wk_sb[:])
    else:
        wq_r, wk_r = wq_sb, wk_sb
    nc.sync.dma_start(idt[:], ident[:])
    nc.sync.dma_start(sbq_sb[:], sbq[:])
    nc.sync.dma_start(bk_sb[:], bk1[:])
    nc.sync.dma_start(bu_sb[:], bass.AP(bu72.tensor, 0, [[0, 128], [1, 72]]))
    nc.sync.dma_start(sel_sb[:], sel[:])

    # ---- long-lived accumulators
    # PT: [to_sq(+ones row), board*64+from_sq] across the whole core batch
    pt = big.tile([65, B_SHARD * 64], F32, tag="pt")
    # promoxt layout: per chunk c (free offset c*1024): [d_in_chunk=128part,
    #   g*64 + b_l*8 + s_l]
    promoxt = big.tile([128, 8 * B_SHARD * 8], F32, tag="promoxt")
    staging = big.tile([128, 64 * 73], F32, tag="staging")

    nc.gpsimd.memset(pt[64:65, :], 1.0)

    # ================= phase 1: projections + attention =================
    with (
        tc.tile_pool(name="xin", bufs=2) as xin_pool,
        tc.tile_pool(name="xtp", bufs=4, space="PSUM") as xtp_pool,
        tc.tile_pool(name="xt", bufs=2) as xt_pool,
        tc.tile_pool(name="qkp", bufs=1, space="PSUM") as qkp_pool,
        tc.tile_pool(name="qkt", bufs=2) as qkt_pool,
        tc.tile_pool(name="app", bufs=2, space="PSUM") as app_pool,
    ):
        for g in range(N_GROUPS):
            # load 512 tokens (8 boards) as one tile [128 tok, 4 subtiles x d]
            xin = xin_pool.tile([128, 4 * D], F32, tag="xin")
            src = xt_flat[g * TOK_G:(g + 1) * TOK_G, :].rearrange(
                "(t p) d -> p t d", p=128)
            nc.sync.dma_start(xin[:].rearrange("p (t d) -> p t d", t=4), src)

            # transpose to xt chunks [d_chunk=128, 512 tok]
            xts = []
            for c in range(8):
                xtp = xtp_pool.tile([128, TOK_G], F32, tag="xtp")
                for t in range(4):
                    nc.tensor.transpose(
                        xtp[:, t * 128:(t + 1) * 128],
                        xin[:, t * D + c * 128:t * D + (c + 1) * 128],
                        idt[:],
                    )
                xt = xt_pool.tile([128, TOK_G], PROJ_DT, tag=f"xt{c}")
                if c % 2 == 0:
                    nc.vector.tensor_copy(xt[:], xtp[:])
                else:
                    nc.scalar.copy(xt[:], xtp[:])
                xts.append(xt)

            # promo-token slices of xT (columns b_l*64 + 48 + s)
            for c in range(8):
                src = xts[c][:].rearrange("p (b s) -> p b s", b=GROUP_B)[:, :, 48:56]
                dst = promoxt[:].rearrange(
                    "p (c g b s) -> p c g b s", c=8, g=N_GROUPS, b=GROUP_B
                )[:, c, g, :, :]
                nc.gpsimd.tensor_copy(dst, src)

            # q/k projections: [p=128, tok]
            qp = qkp_pool.tile([128, TOK_G], F32, tag="qp")
            kp = qkp_pool.tile([128, TOK_G], F32, tag="kp")
            for c in range(8):
                nc.tensor.matmul(qp[:], wq_r[:, c * 128:(c + 1) * 128],
                                 xts[c][:], start=(c == 0), stop=(c == 7))
            for c in range(8):
                nc.tensor.matmul(kp[:], wk_r[:, c * 128:(c + 1) * 128],
                                 xts[c][:], start=(c == 0), stop=(c == 7))
            qt = qkt_pool.tile([128, TOK_G], F32, tag="qt")
            kt = qkt_pool.tile([128, TOK_G], F32, tag="kt")
            nc.vector.tensor_scalar(qt[:], qp[:], SCALE, sbq_sb[:],
                                    op0=mybir.AluOpType.mult,
                                    op1=mybir.AluOpType.add)
            nc.scalar.activation(kt[:], kp[:],
                                 mybir.ActivationFunctionType.Identity,
                                 bias=bk_sb[:], scale=1.0)

            # attention: P^T[t, f] per board
            ap = app_pool.tile([64, TOK_G], F32, tag="ap")
            for b in range(GROUP_B):
                nc.tensor.matmul(ap[:, b * 64:(b + 1) * 64],
                                 kt[:, b * 64:(b + 1) * 64],
                                 qt[:, b * 64:(b + 1) * 64],
                                 start=True, stop=True)
            nc.vector.tensor_copy(pt[0:64, g * TOK_G:(g + 1) * TOK_G], ap[:])

    # ================= phase 2: gather + underpromotions =================
    with (
        tc.tile_pool(name="gp", bufs=3, space="PSUM") as gp_pool,
        tc.tile_pool(name="up", bufs=1, space="PSUM") as up_pool,
    ):
        # underpromotion head first: up[b, s*9+j] = x[b,48+s,:] @ Wu[:,j]
        up = up_pool.tile([128, 72], F32, tag="up")
        for s_l in range(8):
            for c in range(8):
                lhs = promoxt[:].rearrange(
                    "p (c bs s) -> p c bs s", c=8, s=8)[:, c, :, s_l]
                nc.tensor.matmul(up[:, s_l * 9:(s_l + 1) * 9], lhs,
                                 wu_sb[:, c * 9:(c + 1) * 9],
                                 start=(c == 0), stop=(c == 7))

        f_per = 6
        n_fg = (64 + f_per - 1) // f_per          # 11 groups (last has 4)
        for fg in range(n_fg):
            f0 = fg * f_per
            nf = min(f_per, 64 - f0)
            gp = gp_pool.tile([128, f_per * 73], F32, tag="gp")
            for f_l in range(nf):
                f = f0 + f_l
                lhs = pt[:].rearrange("t (b f) -> t b f", f=64)[:, :, f]
                nc.tensor.matmul(gp[:, f_l * 73:(f_l + 1) * 73], lhs,
                                 sel_sb[:, f * 73:(f + 1) * 73],
                                 start=True, stop=True)
            # merge u (+bu) into the promo columns of this psum block
            pf0, pf1 = max(f0, 48), min(f0 + nf, 56)
            if pf0 < pf1:
                dst = gp[:].rearrange("b (f p) -> b f p", p=73)[
                    :, pf0 - f0:pf1 - f0, 64:73]
                src = up[:].rearrange("b (s j) -> b s j", s=8)[
                    :, pf0 - 48:pf1 - 48, :]
                b_ap = bu_sb[:].rearrange("b (s j) -> b s j", s=8)[
                    :, pf0 - 48:pf1 - 48, :]
                nc.vector.tensor_tensor(dst, src, b_ap, op=mybir.AluOpType.add)
            nc.vector.tensor_copy(staging[:, f0 * 73:(f0 + nf) * 73],
                                  gp[:, 0:nf * 73])
            nc.sync.dma_start(out[:, f0 * 73:(f0 + nf) * 73],
                              staging[:, f0 * 73:(f0 + nf) * 73])


def build_nc():
    nc = bacc.Bacc("TRN2", target_bir_lowering=False, debug=False)
    x = nc.dram_tensor("x", [B_SHARD, 64, D], F32, kind="ExternalInput").ap()
    wq = nc.dram_tensor("wq", [D, P], F32, kind="ExternalInput").ap()
    wk = nc.dram_tensor("wk", [D, P], F32, kind="ExternalInput").ap()
    wu = nc.dram_tensor("wu", [D, 9], F32, kind="ExternalInput").ap()
    sbq = nc.dram_tensor("sbq", [P, 1], F32, kind="ExternalInput").ap()
    bk1 = nc.dram_tensor("bk1", [P, 1], F32, kind="ExternalInput").ap()
    bu72 = nc.dram_tensor("bu72", [1, 72], F32, kind="ExternalInput").ap()
    sel = nc.dram_tensor("sel", [65, 64 * 73], F32, kind="ExternalInput").ap()
    ident = nc.dram_tensor("ident", [128, 128], F32, kind="ExternalInput").ap()
    out = nc.dram_tensor("out", [B_SHARD, 64 * 73], F32,
                         kind="ExternalOutput").ap()
    with tile.TileContext(nc) as tc:
        policy_head_kernel(tc, out, x, wq, wk, wu, sbq, bk1, bu72, sel, ident)
    nc.compile()
    return nc


_NC_CACHE = None


def _get_nc():
    global _NC_CACHE
    if _NC_CACHE is None:
        _NC_CACHE = build_nc()
    return _NC_CACHE


def make_in_maps(x, Wq, bq, Wk, bk, Wu, bu):
    sel = _build_sel65()
    ident = np.eye(128, dtype=np.float32)
    sbq = (np.asarray(bq, np.float32) * SCALE).reshape(P, 1)
    bk1 = np.asarray(bk, np.float32).reshape(P, 1)
    bu72 = np.tile(np.asarray(bu, np.float32), 8).reshape(1, 72)
    xs = np.ascontiguousarray(np.asarray(x, np.float32)).reshape(
        N_CORES, B_SHARD, 64, D)
    base = dict(wq=np.asarray(Wq, np.float32), wk=np.asarray(Wk, np.float32),
                wu=np.asarray(Wu, np.float32), sbq=sbq, bk1=bk1, bu72=bu72,
                sel=sel, ident=ident)
    return [dict(base, x=xs[c]) for c in range(N_CORES)]


def kernel(x, Wq, bq, Wk, bk, Wu, bu):
    nc = _get_nc()
    in_maps = make_in_maps(x, Wq, bq, Wk, bk, Wu, bu)
    res = bass_utils.run_bass_kernel_spmd(nc, in_maps,
                                          core_ids=list(range(N_CORES)))
    return np.concatenate([r["out"] for r in res.results], axis=0)


# revision 22
# speedup vs baseline: 1.0531x; 1.0038x over previous
"""Trainium2 Bass kernel for the LC0-style attention policy head.

Computation (per board of 64 squares):
  q = x @ Wq + bq; k = x @ Wk + bk            (P = 128)
  logits_ft = q @ k^T * P**-0.5               (64 x 64)
  logits_64[f, p] = logits_ft[f, to_sq[f, p]] masked to -1e9 where invalid
  up = (x @ Wu + bu) masked to -1e9 except rank-7 rows (48..55)
  out = concat([logits_64, up], -1).reshape(64 * 73)

Sharding: pure data parallel, batch 1024 -> 8 cores x 128.

Key layout ideas:
  * x arrives [tokens, d]; projections need d on partitions, so x blocks are
    transposed on the PE (f32 PE transpose is exact).
  * Attention matmuls produce P^T tiles [to_sq, (board, from_sq)] packed in
    SBUF across the whole core batch.
  * The to-square gather is a per-from-square selection matrix matmul:
    out[b, planes] = PT[:, b*64+f].T @ Sel_f, with boards on PSUM partitions.
    A constant ones-row appended to PT plus a bias row in Sel folds the
    "-1e9 where invalid" mask into the same matmul.
  * The underpromotion head runs on promo tokens only (8 per board), gathered
    during phase 1 into a [d, promo-token] buffer.
"""

import sys

sys.path.insert(0, "/opt/trn_rl_repo")

from contextlib import ExitStack

import numpy as np

import concourse.bass as bass
import concourse.mybir as mybir
import concourse.tile as tile
from concourse import bacc, bass_utils
from concourse._compat import with_exitstack

F32 = mybir.dt.float32
F32R = mybir.dt.float32r
USE_F32R_PROJ = True
PROJ_DT = F32R if USE_F32R_PROJ else F32

N_CORES = 8
B_TOTAL = 1024
B_SHARD = B_TOTAL // N_CORES          # 128 boards per core
GROUP_B = 8                            # boards per phase-1 group
N_GROUPS = B_SHARD // GROUP_B          # 16
TOK_G = GROUP_B * 64                   # 512 tokens per group
D = 1024
P = 128
SCALE = float(P) ** -0.5
NEG_INF = -1e9


def _build_tables():
    to_sq = np.zeros((64, 64), np.int32)
    valid = np.zeros((64, 64), bool)
    dirs = [(1, 0), (1, 1), (0, 1), (-1, 1), (-1, 0), (-1, -1), (0, -1), (1, -1)]
    knights = [(2, 1), (1, 2), (-1, 2), (-2, 1), (-2, -1), (-1, -2), (1, -2), (2, -1)]
    for sq in range(64):
        r, c = divmod(sq, 8)
        for d, (dr, dc) in enumerate(dirs):
            for dist in range(1, 8):
                p = d * 7 + (dist - 1)
                nr, nc = r + dr * dist, c + dc * dist
                if 0 <= nr < 8 and 0 <= nc < 8:
                    to_sq[sq, p] = nr * 8 + nc
                    valid[sq, p] = True
        for i, (dr, dc) in enumerate(knights):
            nr, nc = r + dr, c + dc
            if 0 <= nr < 8 and 0 <= nc < 8:
                to_sq[sq, 56 + i] = nr * 8 + nc
                valid[sq, 56 + i] = True
    return to_sq, valid


def _build_sel65():
    """sel[t, f*73 + p]: selection matrix plus bias row (row 64)."""
    to_sq, valid = _build_tables()
    sel = np.zeros((65, 64 * 73), np.float32)
    for f in range(64):
        for p in range(64):
            if valid[f, p]:
                sel[to_sq[f, p], f * 73 + p] = 1.0
            else:
                sel[64, f * 73 + p] = NEG_INF
        for j in range(9):
            if not (48 <= f < 56):
                sel[64, f * 73 + 64 + j] = NEG_INF
    return sel


@with_exitstack
def policy_head_kernel(ctx: ExitStack, tc: tile.TileContext, out, x, wq, wk, wu,
                       sbq, bk1, bu72, sel, ident):
    nc = tc.nc
    xt_flat = x.flatten_outer_dims()      # [8192, 1024] tokens x d

    const = ctx.enter_context(tc.tile_pool(name="const", bufs=1))
    big = ctx.enter_context(tc.tile_pool(name="big", bufs=1))

    # ---- constants / weights in SBUF
    wq_sb = const.tile([128, D], F32, tag="wq_sb")       # 8 chunks [128,128]
    wk_sb = const.tile([128, D], F32, tag="wk_sb")
    wu_sb = const.tile([128, 72], F32, tag="wu_sb")      # 8 chunks [128,9]
    idt = const.tile([128, 128], F32, tag="idt")
    sbq_sb = const.tile([128, 1], F32, tag="sbq_sb")
    bk_sb = const.tile([128, 1], F32, tag="bk_sb")
    bu_sb = const.tile([128, 72], F32, tag="bu_sb")
    sel_sb = const.tile([65, 64 * 73], F32, tag="sel_sb")
    nc.sync.dma_start(wq_sb[:].rearrange("p (c n) -> p c n", c=8),
                      wq[:].rearrange("(c p) n -> p c n", p=128))
    nc.sync.dma_start(wk_sb[:].rearrange("p (c n) -> p c n", c=8),
                      wk[:].rearrange("(c p) n -> p c n", p=128))
    nc.sync.dma_start(wu_sb[:].rearrange("p (c n) -> p c n", c=8),
                      wu[:].rearrange("(c p) n -> p c n", p=128))
    if USE_F32R_PROJ:
        wq_r = const.tile([128, D], PROJ_DT, tag="wq_r")
        wk_r = const.tile([128, D], PROJ_DT, tag="wk_r")
        nc.vector.tensor_copy(wq_r[:], wq_sb[:])
        nc.vector.tensor_copy(wk_r[:], wk_sb[:])
    else:
        wq_r, wk_r = wq_sb, wk_sb
    nc.sync.dma_start(idt[:], ident[:])
    nc.sync.dma_start(sbq_sb[:], sbq[:])
    nc.sync.dma_start(bk_sb[:], bk1[:])
    nc.sync.dma_start(bu_sb[:], bass.AP(bu72.tensor, 0, [[0, 128], [1, 72]]))
    nc.sync.dma_start(sel_sb[:], sel[:])

    # ---- long-lived accumulators
    # PT: [to_sq(+ones row), board*64+from_sq] across the whole core batch
    pt = big.tile([65, B_SHARD * 64], F32, tag="pt")
    # promoxt layout: per chunk c (free offset c*1024): [d_in_chunk=128part,
    #   g*64 + b_l*8 + s_l]
    promoxt = big.tile([128, 8 * B_SHARD * 8], F32, tag="promoxt")
    staging = big.tile([128, 64 * 73], F32, tag="staging")

    nc.gpsimd.memset(pt[64:65, :], 1.0)

    # ================= phase 1: projections + attention =================
    with (
        tc.tile_pool(name="xin", bufs=2) as xin_pool,
        tc.tile_pool(name="xtp", bufs=3, space="PSUM") as xtp_pool,
        tc.tile_pool(name="xt", bufs=2) as xt_pool,
        tc.tile_pool(name="qkp", bufs=1, space="PSUM") as qkp_pool,
        tc.tile_pool(name="qkt", bufs=2) as qkt_pool,
        tc.tile_pool(name="app", bufs=3, space="PSUM") as app_pool,
    ):
        for g in range(N_GROUPS):
            # load 512 tokens (8 boards) as one tile [128 tok, 4 subtiles x d]
            xin = xin_pool.tile([128, 4 * D], F32, tag="xin")
            src = xt_flat[g * TOK_G:(g + 1) * TOK_G, :].rearrange(
                "(t p) d -> p t d", p=128)
            nc.sync.dma_start(xin[:].rearrange("p (t d) -> p t d", t=4), src)

            # transpose to xt chunks [d_chunk=128, 512 tok]
            xts = []
            for c in range(8):
                xtp = xtp_pool.tile([128, TOK_G], F32, tag="xtp")
                for t in range(4):
                    nc.tensor.transpose(
                        xtp[:, t * 128:(t + 1) * 128],
                        xin[:, t * D + c * 128:t * D + (c + 1) * 128],
                        idt[:],
                    )
                xt = xt_pool.tile([128, TOK_G], PROJ_DT, tag=f"xt{c}")
                if c % 2 == 0:
                    nc.vector.tensor_copy(xt[:], xtp[:])
                else:
                    nc.scalar.copy(xt[:], xtp[:])
                xts.append(xt)

            # promo-token slices of xT (columns b_l*64 + 48 + s)
            for c in range(8):
                src = xts[c][:].rearrange("p (b s) -> p b s", b=GROUP_B)[:, :, 48:56]
                dst = promoxt[:].rearrange(
                    "p (c g b s) -> p c g b s", c=8, g=N_GROUPS, b=GROUP_B
                )[:, c, g, :, :]
                nc.gpsimd.tensor_copy(dst, src)

            # q/k projections: [p=128, tok]
            qp = qkp_pool.tile([128, TOK_G], F32, tag="qp")
            kp = qkp_pool.tile([128, TOK_G], F32, tag="kp")
            for c in range(8):
                nc.tensor.matmul(qp[:], wq_r[:, c * 128:(c + 1) * 128],
                                 xts[c][:], start=(c == 0), stop=(c == 7))
            for c in range(8):
                nc.tensor.matmul(kp[:], wk_r[:, c * 128:(c + 1) * 128],
                                 xts[c][:], start=(c == 0), stop=(c == 7))
            qt = qkt_pool.tile([128, TOK_G], F32, tag="qt")
            kt = qkt_pool.tile([128, TOK_G], F32, tag="kt")
            nc.vector.tensor_scalar(qt[:], qp[:], SCALE, sbq_sb[:],
                                    op0=mybir.AluOpType.mult,
                                    op1=mybir.AluOpType.add)
            nc.scalar.activation(kt[:], kp[:],
                                 mybir.ActivationFunctionType.Identity,
                                 bias=bk_sb[:], scale=1.0)

            # attention: P^T[t, f] per board
            ap = app_pool.tile([64, TOK_G], F32, tag="ap")
            for b in range(GROUP_B):
                nc.tensor.matmul(ap[:, b * 64:(b + 1) * 64],
                                 kt[:, b * 64:(b + 1) * 64],
                                 qt[:, b * 64:(b + 1) * 64],
                                 start=True, stop=True)
            nc.vector.tensor_copy(pt[0:64, g * TOK_G:(g + 1) * TOK_G], ap[:])

    # ================= phase 2: gather + underpromotions =================
    with (
        tc.tile_pool(name="gp", bufs=3, space="PSUM") as gp_pool,
        tc.tile_pool(name="up", bufs=1, space="PSUM") as up_pool,
    ):
        # underpromotion head first: up[b, s*9+j] = x[b,48+s,:] @ Wu[:,j]
        up = up_pool.tile([128, 72], F32, tag="up")
        for s_l in range(8):
            for c in range(8):
                lhs = promoxt[:].rearrange(
                    "p (c bs s) -> p c bs s", c=8, s=8)[:, c, :, s_l]
                nc.tensor.matmul(up[:, s_l * 9:(s_l + 1) * 9], lhs,
                                 wu_sb[:, c * 9:(c + 1) * 9],
                                 start=(c == 0), stop=(c == 7))

        f_per = 6
        n_fg = (64 + f_per - 1) // f_per          # 11 groups (last has 4)
        for fg in range(n_fg):
            f0 = fg * f_per
            nf = min(f_per, 64 - f0)
            gp = gp_pool.tile([128, f_per * 73], F32, tag="gp")
            for f_l in range(nf):
                f = f0 + f_l
                lhs = pt[:].rearrange("t (b f) -> t b f", f=64)[:, :, f]
                nc.tensor.matmul(gp[:, f_l * 73:(f_l + 1) * 73], lhs,
                                 sel_sb[:, f * 73:(f + 1) * 73],
                                 start=True, stop=True)
            # merge u (+bu) into the promo columns of this psum block
            pf0, pf1 = max(f0, 48), min(f0 + nf, 56)
            if pf0 < pf1:
                dst = gp[:].rearrange("b (f p) -> b f p", p=73)[
                    :, pf0 - f0:pf1 - f0, 64:73]
                src = up[:].rearrange("b (s j) -> b s j", s=8)[
                    :, pf0 - 48:pf1 - 48, :]
                b_ap = bu_sb[:].rearrange("b (s j) -> b s j", s=8)[
                    :, pf0 - 48:pf1 - 48, :]
                nc.vector.tensor_tensor(dst, src, b_ap, op=mybir.AluOpType.add)
            nc.vector.tensor_copy(staging[:, f0 * 73:(f0 + nf) * 73],
                                  gp[:, 0:nf * 73])
            nc.sync.dma_start(out[:, f0 * 73:(f0 + nf) * 73],
                              staging[:, f0 * 73:(f0 + nf) * 73])


def build_nc():
    nc = bacc.Bacc("TRN2", target_bir_lowering=False, debug=False)
    x = nc.dram_tensor("x", [B_SHARD, 64, D], F32, kind="ExternalInput").ap()
    wq = nc.dram_tensor("wq", [D, P], F32, kind="ExternalInput").ap()
    wk = nc.dram_tensor("wk", [D, P], F32, kind="ExternalInput").ap()
    wu = nc.dram_tensor("wu", [D, 9], F32, kind="ExternalInput").ap()
    sbq = nc.dram_tensor("sbq", [P, 1], F32, kind="ExternalInput").ap()
    bk1 = nc.dram_tensor("bk1", [P, 1], F32, kind="ExternalInput").ap()
    bu72 = nc.dram_tensor("bu72", [1, 72], F32, kind="ExternalInput").ap()
    sel = nc.dram_tensor("sel", [65, 64 * 73], F32, kind="ExternalInput").ap()
    ident = nc.dram_tensor("ident", [128, 128], F32, kind="ExternalInput").ap()
    out = nc.dram_tensor("out", [B_SHARD, 64 * 73], F32,
                         kind="ExternalOutput").ap()
    with tile.TileContext(nc) as tc:
        policy_head_kernel(tc, out, x, wq, wk, wu, sbq, bk1, bu72, sel, ident)
    nc.compile()
    return nc


_NC_CACHE = None


def _get_nc():
    global _NC_CACHE
    if _NC_CACHE is None:
        _NC_CACHE = build_nc()
    return _NC_CACHE


def make_in_maps(x, Wq, bq, Wk, bk, Wu, bu):
    sel = _build_sel65()
    ident = np.eye(128, dtype=np.float32)
    sbq = (np.asarray(bq, np.float32) * SCALE).reshape(P, 1)
    bk1 = np.asarray(bk, np.float32).reshape(P, 1)
    bu72 = np.tile(np.asarray(bu, np.float32), 8).reshape(1, 72)
    xs = np.ascontiguousarray(np.asarray(x, np.float32)).reshape(
        N_CORES, B_SHARD, 64, D)
    base = dict(wq=np.asarray(Wq, np.float32), wk=np.asarray(Wk, np.float32),
                wu=np.asarray(Wu, np.float32), sbq=sbq, bk1=bk1, bu72=bu72,
                sel=sel, ident=ident)
    return [dict(base, x=xs[c]) for c in range(N_CORES)]


def kernel(x, Wq, bq, Wk, bk, Wu, bu):
    nc = _get_nc()
    in_maps = make_in_maps(x, Wq, bq, Wk, bk, Wu, bu)
    res = bass_utils.run_bass_kernel_spmd(nc, in_maps,
                                          core_ids=list(range(N_CORES)))
    return np.concatenate([r["out"] for r in res.results], axis=0)
